# revision 54
# baseline (speedup 1.0000x reference)
"""Trainium2 Bass kernel for nn_Attention_27797028340174.

Multi-head attention, B=4, S=2048, H=16 heads, D=64 (HID=1024):
    x = query.reshape(B*S, HID)                     (the `key` input is
    q,k,v = x@Wq+bq, x@Wk+bk, x@Wv+bv                ignored: source bug
    per (b,h): softmax(q k^T / 8) @ v                makes k,v from query)

Sharding: tensor-parallel over the 16 heads -> 2 heads per NeuronCore,
zero collectives. Each core receives the full transposed activations
xT = x.T (bf16) plus its 128-column slice of Wq/Wk/Wv, and produces its
[8192, 128] slice of the output; the host concatenates slices.

v4 design notes (delta vs v3). The scores matmuls contract over D=64
only; v3 padded them to K=128 (half the PE array multiplied zeros).
v4 runs them as 64-row matmuls in ALTERNATING partition halves: even
j-tiles' kT live in SBUF partitions 0:63, odd j-tiles' in 64:127, and
qT is duplicated into both halves. Consecutive even/odd matmuls then
occupy disjoint PE row-groups and overlap almost fully (measured
~110 ns per 512-wide MM vs ~240 ns unpaired), halving scores PE time.
Per j-tile-PAIR (jp): ic0/ic1 of both parities interleave into one
[128,2048] psum (each 512-chunk = one bank) consumed by ONE wide exp
ACT whose 4D out-AP de-interleaves into the pair's exp tile; ic2/ic3
go through two [128,1024] narrow psums/ACTs. ACT count and PSUM bank
budget match v3 (the Scalar engine exp stream stays the pacer at
~32 us/pair); the PE gains ~8 us/pair of slack which absorbs the
ctx/proj/v units without fills running the clock-warmup risk.

Device algorithm per core (all matmuls bf16, fp32 PSUM):
  qT,kT = W.T @ xT   transposed activations, 64 valid rows per parity
  v     = xT.T @ Wv  [seq, 128] natural layout (+ ones column)
  per (b,h), per jp: scoresT = kT_half.T @ qT_half  (K=64, paired)
      expT = exp(scoresT/8)  (ScalarE, bf16 out)
  per i-group of 4 tiles: ctx, Z = expT.T @ [v | 1]  (K=128 chains)
      out = ctx * reciprocal(Z)    (VectorE)

Assumptions hard-verified on host: attention_mask all ones (mask term
== 0), zero biases. These hold for the problem's setup_inputs().
"""

from contextlib import ExitStack

import numpy as np
import ml_dtypes

import concourse.bass as bass
import concourse.tile as tile
from concourse import bacc, mybir
from concourse.bass_utils import run_bass_kernel_spmd

BF16 = mybir.dt.bfloat16
F32 = mybir.dt.float32

B = 4  # batches
S = 2048  # seq per batch
S2 = 1024  # kT cols per head (16 j-tiles packed as 8 parity pairs)
HID = 1024
NCORES = 8
NH = 2  # heads per core
D = 64
KT = 8  # hid tiles of 128
JT = 16  # key tiles of 128 per batch
JP = 8  # key tile pairs per batch
IT = 16  # query tiles of 128 per batch
CH = 4  # seq chunks of 512 per batch
CW = 512  # chunk width

EXP_BUFS = 16  # [128, 2*S] pair tiles
XT_BUFS = 3

_CACHE = {}


def _build():
    nc = bacc.Bacc(
        "TRN2", target_bir_lowering=False, debug=False, num_devices=NCORES
    )
    xt = nc.dram_tensor("xt", [HID, B * S], BF16, kind="ExternalInput")
    wq = nc.dram_tensor("wq", [HID, 128], BF16, kind="ExternalInput")
    wk = nc.dram_tensor("wk", [HID, 128], BF16, kind="ExternalInput")
    wv = nc.dram_tensor("wv", [HID, 128], BF16, kind="ExternalInput")
    out = nc.dram_tensor("out", [B * S, 128], F32, kind="ExternalOutput")

    xt_v = xt.ap().rearrange("(kt p) n -> p kt n", p=128)  # [128, 8, 8192]
    out_v = out.ap().rearrange("(b it p) c -> p b it c", it=IT, p=128)

    with tile.TileContext(nc) as tc, ExitStack() as ctx:
        wp = ctx.enter_context(tc.tile_pool(name="w", bufs=1))
        xp = ctx.enter_context(tc.tile_pool(name="x", bufs=XT_BUFS))
        qkp = ctx.enter_context(tc.tile_pool(name="qk", bufs=2))
        ep = ctx.enter_context(tc.tile_pool(name="e", bufs=EXP_BUFS))
        op = ctx.enter_context(tc.tile_pool(name="o", bufs=2))
        zp = ctx.enter_context(tc.tile_pool(name="z", bufs=4))
        ksp = ctx.enter_context(tc.tile_pool(name="ks", bufs=3))
        # 8 PSUM banks: scores wide [128,2048] (4) + narrow [128,1024] (2)
        # + proj/v chain bank (1) + ctx chain bank (1).  ctx chains stay
        # open across other units, and a chain start= clears its whole
        # 2KB bank -- so ctx must NEVER share a bank with proj/v chains.
        psW = ctx.enter_context(tc.tile_pool(name="psW", bufs=1, space="PSUM"))
        psN = ctx.enter_context(tc.tile_pool(name="psN", bufs=1, space="PSUM"))
        psq = ctx.enter_context(tc.tile_pool(name="psq", bufs=1, space="PSUM"))
        psc = ctx.enter_context(tc.tile_pool(name="psc", bufs=1, space="PSUM"))

        wq_sb = wp.tile([128, KT, 128], BF16)
        nc.sync.dma_start(wq_sb[:], wq.ap().rearrange("(kt p) m -> p kt m", p=128))
        wk_sb = wp.tile([128, KT, 128], BF16)
        wv_sb = wp.tile([128, KT, 128], BF16)
        w_loaded = {"k": False, "v": False}

        def load_w(which):
            if not w_loaded[which]:
                src = wk if which == "k" else wv
                dst = wk_sb if which == "k" else wv_sb
                eng = nc.scalar if which == "k" else nc.sync
                eng.dma_start(
                    dst[:], src.ap().rearrange("(kt p) m -> p kt m", p=128)
                )
                w_loaded[which] = True

        state = {}
        xt_carry = {}  # (batch, chunk) -> in-flight xt SBUF tile

        # qT: [128, NH*S], head h at cols h*S, rows 0:64 and 64:128 hold
        # the SAME data (duplicated so even/odd scores matmuls can read
        # from either partition half).  kT: [128, NH*S2], head h at cols
        # h*S2; j-tile pair jp at cols jp*128; rows 0:64 = even j-tile
        # (2jp), rows 64:128 = odd j-tile (2jp+1).
        qT_bufs = [wp.tile([128, NH * S], BF16, name=f"qTb{i}") for i in (0, 1)]
        kT_bufs = [wp.tile([128, NH * S2], BF16, name=f"kTb{i}") for i in (0, 1)]

        def alloc_batch(b):
            st = {}
            st["qT"] = qT_bufs[b % 2]
            st["kT"] = kT_bufs[b % 2]
            st["v"] = qkp.tile([128, JT, NH, D + 1], BF16, tag="v", name="v")
            nc.vector.memset(st["v"][:, :, :, D], 1.0)
            state[b] = st

        # ---- projection emitters ----
        def _q_copyout(b, ch, ps, ring=None):
            qT = state[b]["qT"]
            eng = nc.scalar if ring == "scalar" else nc.sync
            c0 = slice(ch * CW, (ch + 1) * CW)
            c1 = slice(S + ch * CW, S + (ch + 1) * CW)
            nc.vector.tensor_copy(out=qT[0:64, c0], in_=ps[0:64, :])
            eng.dma_start(qT[64:128, c0], qT[0:64, c0])
            nc.vector.tensor_copy(out=qT[64:128, c1], in_=ps[64:128, :])
            eng.dma_start(qT[0:64, c1], qT[64:128, c1])

        def _k_copyout(b, ch, ps, ring=None):
            kT = state[b]["kT"]
            eng = nc.scalar if ring == "scalar" else nc.sync
            base = 2 * ch * 128  # 256-col pair block within a head region
            psv0 = ps[0:64, :].rearrange("p (jp pr c) -> p jp pr c", pr=2, c=128)
            psv1 = ps[64:128, :].rearrange("p (jp pr c) -> p jp pr c", pr=2, c=128)
            stg = ksp.tile([128, 256], BF16, tag="kstg", name="kstg")
            k0e = kT[0:64, base : base + 256].rearrange(
                "p (jp c) -> p jp c", c=128
            )
            nc.vector.tensor_copy(out=k0e, in_=psv0[:, :, 0])
            stg0 = stg[0:64, :].rearrange("p (jp c) -> p jp c", c=128)
            nc.vector.tensor_copy(out=stg0, in_=psv0[:, :, 1])
            eng.dma_start(kT[64:128, base : base + 256], stg[0:64, :])
            k1o = kT[64:128, S2 + base : S2 + base + 256].rearrange(
                "p (jp c) -> p jp c", c=128
            )
            nc.vector.tensor_copy(out=k1o, in_=psv1[:, :, 1])
            stg1 = stg[64:128, :].rearrange("p (jp c) -> p jp c", c=128)
            nc.vector.tensor_copy(out=stg1, in_=psv1[:, :, 0])
            eng.dma_start(kT[0:64, S2 + base : S2 + base + 256], stg[64:128, :])

        def load_xt(b, ch, ring=None):
            gc = b * CH + ch
            xt_t = xp.tile([128, KT, CW], BF16, tag="xt", name="xt_t")
            if ring == "split":
                # halve latency: kt 0:4 on the sync ring, 4:8 on scalar
                nc.sync.dma_start(
                    xt_t[:, 0:4], xt_v[:, 0:4, gc * CW : (gc + 1) * CW]
                )
                nc.scalar.dma_start(
                    xt_t[:, 4:KT], xt_v[:, 4:KT, gc * CW : (gc + 1) * CW]
                )
                return xt_t
            eng = nc.scalar if ring == "scalar" else nc.sync
            eng.dma_start(xt_t[:], xt_v[:, :, gc * CW : (gc + 1) * CW])
            return xt_t

        def proj_half1(b, ch, which, xt_t, pool=None):
            """First half of a q/k projection chunk: open the chain."""
            if which == "k":
                load_w("k")
            w_sb = wq_sb if which == "q" else wk_sb
            ps = (pool or psq).tile(
                [128, CW], F32,
                tag="aux" if (pool or psq) is psq else "ctx", name="ps_qk")
            for kt in range(4):
                nc.tensor.matmul(
                    ps[:], lhsT=w_sb[:, kt], rhs=xt_t[:, kt],
                    start=(kt == 0), stop=False,
                )
            return ps

        def proj_half2(b, ch, which, xt_t, ps, ring=None):
            w_sb = wq_sb if which == "q" else wk_sb
            for kt in range(4, KT):
                nc.tensor.matmul(
                    ps[:], lhsT=w_sb[:, kt], rhs=xt_t[:, kt],
                    start=False, stop=(kt == KT - 1),
                )
            if which == "q":
                _q_copyout(b, ch, ps, ring=ring)
            else:
                _k_copyout(b, ch, ps, ring=ring)

        def emit_proj(b, ch, which, xt_t, pool=None, ring=None):
            """Full q or k projection chunk (8 matmuls + copyout)."""
            ps = proj_half1(b, ch, which, xt_t, pool=pool)
            proj_half2(b, ch, which, xt_t, ps, ring=ring)

        def emit_v_half(b, ch, half, xt_t):
            load_w("v")
            st = state[b]
            ps_full = psq.tile([128, CW], F32, tag="aux", name="ps_v")
            ps = ps_full[:, 0:256]
            for si, sub in enumerate((2 * half, 2 * half + 1)):
                for kt in range(KT):
                    nc.tensor.matmul(
                        ps[:, si * 128 : (si + 1) * 128],
                        lhsT=xt_t[:, kt, sub * 128 : (sub + 1) * 128],
                        rhs=wv_sb[:, kt],
                        start=(si == 0 and kt == 0),
                        stop=(si == 1 and kt == KT - 1),
                    )
            nc.vector.tensor_copy(
                out=st["v"][:, ch * 4 + 2 * half : ch * 4 + 2 * half + 2, :, 0:D],
                in_=ps[:].rearrange("p (s h d) -> p s h d", s=2, h=NH),
            )

        # ---- scores emitters ----
        def _smm(p, jt, ic, ps_dst):
            b, h = divmod(p, NH)
            st = state[b]
            jp, pr = divmod(jt, 2)
            half = slice(0, 64) if pr == 0 else slice(64, 128)
            nc.tensor.matmul(
                ps_dst,
                lhsT=st["kT"][half, h * S2 + jp * 128 : h * S2 + (jp + 1) * 128],
                rhs=st["qT"][half, h * S + ic * CW : h * S + (ic + 1) * CW],
                start=True,
                stop=True,
            )

        def _act(e_slice, ps_slice):
            nc.scalar.activation(
                e_slice, ps_slice, mybir.ActivationFunctionType.Exp, scale=0.125
            )

        def _eo(e):
            # [128, 4*512 per parity] -> [p, ic(4), parity(2), 512]
            return e[:].rearrange("p (t a c) -> p a t c", t=2, a=4)

        def emit_W_mms(p, jp):
            """ics 0,1 of both parities interleaved into one [128,2048]."""
            psw = psW.tile([128, 2048], F32, tag="sW", name="ps_w")
            for ic in (0, 1):
                for pr in (0, 1):
                    _smm(p, 2 * jp + pr, ic,
                         psw[:, (2 * ic + pr) * CW : (2 * ic + pr + 1) * CW])
            return psw

        def emit_W_act(psw, e):
            inv = psw[:].rearrange("p (a t c) -> p a t c", a=2, t=2)
            _act(_eo(e)[:, 0:2], inv)

        def emit_N_mms(p, jp, ic, pool=None):
            pool = pool or psN
            psn = pool.tile([128, 1024], F32,
                            tag="sN" if pool is psN else "sW", name="ps_n")
            for pr in (0, 1):
                _smm(p, 2 * jp + pr, ic, psn[:, pr * CW : (pr + 1) * CW])
            return psn

        def emit_N_act(psn, ic, e):
            inv = psn[:].rearrange("p (t c) -> p t c", t=2)
            _act(_eo(e)[:, ic], inv)

        def jp_W(p, jp, e):
            emit_W_act(emit_W_mms(p, jp), e)

        pro_npool = {"i": 0}

        def jp_N(p, jp, ic, e):
            """Prologue narrow group; alternates the psN / psW banks so
            consecutive narrow ACTs never wait each other's psum."""
            pool = psN if pro_npool["i"] % 2 == 0 else psW
            pro_npool["i"] += 1
            emit_N_act(emit_N_mms(p, jp, ic, pool=pool), ic, e)

        def mk_exps(pts):
            return [pts[j // 2][:, (j % 2) * S : (j % 2) * S + S] for j in range(JT)]

        # ---- ctx emitters ----
        def ctx_quartet(p, it0, exps, jts, cstate):
            """4 j-tiles x 4 q-slices of an accumulation chain."""
            b, h = divmod(p, NH)
            st = state[b]
            W = D + 1
            if cstate["ps"] is None:
                cstate["ps"] = psc.tile([128, 4 * W], F32, tag="ctx", name="ps_c")
            ps = cstate["ps"]
            for jt in jts:
                for q in range(4):
                    cstate["n"] += 1
                    nc.tensor.matmul(
                        ps[:, q * W : (q + 1) * W],
                        lhsT=exps[jt][:, (it0 + q) * 128 : (it0 + q + 1) * 128],
                        rhs=st["v"][:, jt, h],
                        start=(cstate["n"] == 1),
                        stop=(cstate["n"] == cstate["tot"]),
                    )
            return ps

        def ctx_norm(ps, it0, o_sb):
            W = D + 1
            rz = zp.tile([128, 4], F32, tag="rz", name="rz")
            z_view = ps[:].rearrange("p (q w) -> p q w", w=W)[:, :, D]
            nc.vector.reciprocal(rz[:], z_view)
            for q in range(4):
                nc.vector.tensor_scalar_mul(
                    o_sb[:, it0 + q], ps[:, q * W : q * W + D], rz[:, q : q + 1]
                )

        def emit_out_dma(p, o_sb, groups=(0, 1, 2, 3)):
            b, h = divmod(p, NH)
            for g in groups:
                nc.sync.dma_start(
                    out_v[:, b, 4 * g : 4 * g + 4, h * D : (h + 1) * D],
                    o_sb[:, 4 * g : 4 * g + 4],
                )

        # =================== prologue ===================
        warm_src = wp.tile([128, 1], F32, name="warm_src")
        warm_dst = wp.tile([128, 1], F32, name="warm_dst")
        nc.vector.memset(warm_src[:], 0.0)
        nc.scalar.activation(
            warm_dst[:], warm_src[:], mybir.ActivationFunctionType.Exp
        )
        # PE clock ramp warmup while the first DMAs fly
        warm_w = wp.tile([128, 128], BF16, name="warm_w")
        nc.vector.memset(warm_w[:], 0.0)
        warm_r = wp.tile([128, CW], BF16, name="warm_r")
        nc.vector.memset(warm_r[:], 0.0)
        warm_ps_full = psq.tile([128, CW], F32, tag="aux", name="warm_ps")
        warm_ps = warm_ps_full[:, 0:64]
        NWARM = 64
        for r in range(NWARM):
            nc.tensor.matmul(
                warm_ps[:, 0:64],
                lhsT=warm_w[:],
                rhs=warm_r[:, 0:64],
                start=(r == 0),
                stop=(r == NWARM - 1),
            )
        warm_junk = wp.tile([128, 64], F32, name="warm_junk")
        nc.vector.tensor_copy(out=warm_junk[:], in_=warm_ps[:, 0:64])
        # second warm chain: keep the PE busy (HAM stays at 8/8) while
        # the xt0 DMA is still in flight; sized to end just before the
        # transfer completes (~16us in)
        warm_ps2_full = psc.tile([128, CW], F32, tag="ctx", name="warm_ps2")
        warm_ps2 = warm_ps2_full[:, 0:64]
        NWARM2 = 84
        for r in range(NWARM2):
            nc.tensor.matmul(
                warm_ps2[:, 0:64],
                lhsT=warm_w[:],
                rhs=warm_r[:, 0:64],
                start=(r == 0),
                stop=(r == NWARM2 - 1),
            )
        nc.vector.tensor_copy(out=warm_junk[:], in_=warm_ps2[:, 0:64])

        alloc_batch(0)
        pts0 = [ep.tile([128, 2 * S], BF16, tag="e", name="e0") for _ in range(JP)]
        exps0 = mk_exps(pts0)

        # batch-0 projections laddered with pair-0 scores; alternate the
        # chain bank psq/psc so chain N+1 never waits chain N's copyout
        pro_pool = {"i": 0}

        def pproj(b, ch, which, xt_t, ring=None):
            pool = psq if pro_pool["i"] % 2 == 0 else psc
            pro_pool["i"] += 1
            emit_proj(b, ch, which, xt_t, pool=pool, ring=ring)

        # ladder: initial DMAs fly on four separate queues (xt0 sync,
        # wq scalar, wk vector, xt1 tensor); the first scores land as
        # narrow 1024-wide ACTs as soon as q0+k0 are projected, so the
        # Scalar engine starts its exp stream ~15us earlier than a
        # wide-first ladder would allow.
        xt_t0 = load_xt(0, 0, ring="split")
        load_w("k")
        xt_t1 = load_xt(0, 1, ring="split")
        pproj(0, 0, "q", xt_t0, ring="scalar")
        pproj(0, 0, "k", xt_t0, ring="scalar")
        jp_N(0, 0, 0, pts0[0])
        jp_N(0, 1, 0, pts0[1])
        pproj(0, 1, "q", xt_t1, ring="scalar")
        jp_N(0, 0, 1, pts0[0])
        jp_N(0, 1, 1, pts0[1])
        xt_t2 = load_xt(0, 2)
        pproj(0, 1, "k", xt_t1, ring="scalar")
        jp_N(0, 2, 0, pts0[2])
        jp_N(0, 3, 0, pts0[3])
        pproj(0, 2, "q", xt_t2)
        jp_N(0, 2, 1, pts0[2])
        jp_N(0, 3, 1, pts0[3])
        jp_N(0, 0, 2, pts0[0])
        pproj(0, 2, "k", xt_t2)
        jp_N(0, 1, 2, pts0[1])
        jp_N(0, 2, 2, pts0[2])
        xt_t3 = load_xt(0, 3, ring="tensor")
        jp_W(0, 4, pts0[4])
        pproj(0, 3, "q", xt_t3)
        jp_N(0, 3, 2, pts0[3])
        jp_N(0, 4, 2, pts0[4])
        jp_N(0, 0, 3, pts0[0])
        pproj(0, 3, "k", xt_t3)
        jp_N(0, 1, 3, pts0[1])
        jp_N(0, 2, 3, pts0[2])
        jp_N(0, 3, 3, pts0[3])
        load_w("v")
        xtv0 = load_xt(0, 0, ring="scalar")
        jp_W(0, 5, pts0[5])
        jp_N(0, 4, 3, pts0[4])
        jp_N(0, 5, 2, pts0[5])
        xtv1 = load_xt(0, 1, ring="scalar")
        emit_v_half(0, 0, 0, xtv0)
        jp_W(0, 6, pts0[6])
        emit_v_half(0, 0, 1, xtv0)
        jp_N(0, 5, 3, pts0[5])
        jp_N(0, 6, 2, pts0[6])
        xtv2 = load_xt(0, 2, ring="scalar")
        emit_v_half(0, 1, 0, xtv1)
        jp_W(0, 7, pts0[7])
        emit_v_half(0, 1, 1, xtv1)
        jp_N(0, 6, 3, pts0[6])
        jp_N(0, 7, 2, pts0[7])
        xtv3 = load_xt(0, 3, ring="scalar")
        emit_v_half(0, 2, 0, xtv2)
        jp_N(0, 7, 3, pts0[7])
        emit_v_half(0, 2, 1, xtv2)
        # pair 1's units consume these: v(0,3) halves + batch-1 chunk-0/1
        xt_carry[(0, 3)] = xtv3
        xt_carry[(1, 0)] = load_xt(1, 0, ring="scalar")
        xt_carry[(1, 1)] = load_xt(1, 1, ring="scalar")
        prev = (0, exps0)

        # =================== steady pairs ===================
        NP = B * NH
        cst_tiles = []
        pend_scores = None

        def build_units(p):
            """PE work units (est_ns, fn, min_grp) for pair p's slots.
            Units are fine-grained (ctx quartets ~650ns, projection
            halves ~880ns) so a pop between ACT groups never injects a
            multi-us block of foreign matmuls ahead of the next scores
            matmuls in the PE stream.  Constraints encoded in the hand
            ordering: a proj chain's two halves have no other psq user
            between them (psq is single banked); consecutive psq chains
            are separated by a ctx quartet so chain N+1's start never
            waits on chain N's copyout; ctx chain g+1 opens only after
            chain g's norm (they share the psc bank); each xt DMA is
            issued >=4 units (and usually a pair boundary) before its
            consumer so an in-flight transfer never head-of-line blocks
            the in-order PE queue."""
            b, h = divmod(p, NH)
            pp, pexps = prev
            o_prev = op.tile([128, IT, D], F32, tag="o", name="o_sb")

            cstates = [{"ps": None, "n": 0, "tot": 64} for _ in range(4)]

            def mk_cq(g, jq):
                """Half-chain unit: 8 j-tiles x 4 q-slices (32 matmuls).
                Longer bursts amortize the PE pipeline-refill cost that
                each ctx burst start pays (~140ns)."""
                def cfn(g=g, jq=jq):
                    ctx_quartet(pp, 4 * g, pexps,
                                range(8 * jq, 8 * jq + 8), cstates[g])
                return (1180, cfn, 0)

            def mk_cn(g):
                def nfn(g=g):
                    ctx_norm(cstates[g]["ps"], 4 * g, o_prev)
                return (90, nfn, 0)

            def dfn():
                emit_out_dma(pp, o_prev)

            pschain = {}

            def mk_ld(bb, chn):
                def ld(bb=bb, chn=chn):
                    xt_carry[(bb, chn)] = load_xt(bb, chn)
                return (30, ld, 0)

            def mk_vh(bb, chn, half):
                def vh(bb=bb, chn=chn, half=half):
                    emit_v_half(bb, chn, half, xt_carry[(bb, chn)])
                return (890, vh, 0)

            def mk_qk1(bb, chn, which):
                def pj(bb=bb, chn=chn, which=which):
                    pschain[which] = proj_half1(
                        bb, chn, which, xt_carry[(bb, chn)]
                    )
                return (880, pj, 0)

            def mk_qk2(bb, chn, which):
                def pj(bb=bb, chn=chn, which=which):
                    proj_half2(bb, chn, which, xt_carry[(bb, chn)],
                               pschain[which])
                return (880, pj, 0)

            def mk_alloc(bb):
                def al(bb=bb):
                    alloc_batch(bb)
                return (10, al, 0)

            def mk_fill(n):
                def fl(n=n):
                    ps = psq.tile([128, CW], F32, tag="aux", name="ps_fill")
                    for r in range(n):
                        nc.tensor.matmul(
                            ps[:],
                            lhsT=warm_w[:],
                            rhs=warm_r[:],
                            start=(r == 0),
                            stop=(r == n - 1),
                        )
                return (n * 215, fl, 0)

            def Cq(g):
                return [mk_cq(g, jq) for jq in range(2)]

            c0, c1, c2, c3 = Cq(0), Cq(1), Cq(2), Cq(3)
            units = []
            if h == 0 and b >= 1:
                # v(b) all chunks + qk(b+1) chunk0; (b,0) and (b,1) were
                # prefetched by the previous pair's tail
                units += [mk_vh(b, 0, 0), c0[0], mk_vh(b, 0, 1),
                          mk_ld(b, 2),
                          mk_vh(b, 1, 0), c0[1], mk_cn(0), mk_vh(b, 1, 1),
                          mk_ld(b, 3),
                          mk_vh(b, 2, 0), c1[0], mk_vh(b, 2, 1)]
                if b + 1 < B:
                    units += [mk_alloc(b + 1), mk_ld(b + 1, 0)]
                units += [mk_vh(b, 3, 0), c1[1], mk_cn(1), mk_vh(b, 3, 1)]
                if b + 1 < B:
                    units += [mk_qk1(b + 1, 0, "q"), c2[0],
                              mk_qk2(b + 1, 0, "q"), c2[1],
                              mk_qk1(b + 1, 0, "k"), mk_cn(2),
                              mk_qk2(b + 1, 0, "k"),
                              c3[0], c3[1], mk_cn(3),
                              (20, dfn, 0),
                              # prefetch c1 for the following h1 pair
                              mk_ld(b + 1, 1)]
                else:
                    units += [c2[0], mk_fill(4), c2[1],
                              mk_cn(2), mk_fill(4),
                              c3[0], mk_fill(4), c3[1],
                              mk_cn(3), (20, dfn, 0), mk_fill(4)]
            elif h == 1 and b + 1 < B:
                bb = b + 1
                if b == 0:
                    # pair 1 absorbs the prologue tail: v(0,3) halves,
                    # batch-1 alloc and its chunk-0 projections
                    units += [mk_vh(0, 3, 0), c0[0], mk_vh(0, 3, 1),
                              mk_alloc(1),
                              mk_qk1(1, 0, "q"), c0[1],
                              mk_qk2(1, 0, "q"), mk_cn(0),
                              mk_qk1(1, 0, "k"), c1[0],
                              mk_qk2(1, 0, "k")]
                else:
                    units += [c0[0], c0[1], mk_cn(0),
                              c1[0]]
                # chunks 1-3 of batch bb; (bb,1) prefetched earlier
                units += [mk_qk1(bb, 1, "q"), mk_ld(bb, 2),
                          mk_qk2(bb, 1, "q"), c1[1], mk_cn(1),
                          mk_qk1(bb, 1, "k"), c2[0],
                          mk_qk2(bb, 1, "k"),
                          mk_qk1(bb, 2, "q"), mk_ld(bb, 3),
                          mk_qk2(bb, 2, "q"), c2[1], mk_cn(2),
                          mk_qk1(bb, 2, "k"), c3[0],
                          mk_qk2(bb, 2, "k"),
                          mk_qk1(bb, 3, "q"), c3[1],
                          mk_qk2(bb, 3, "q"), mk_cn(3),
                          mk_qk1(bb, 3, "k"), (20, dfn, 0),
                          mk_qk2(bb, 3, "k"),
                          # prefetch v0/v1 chunks for the next h0 pair
                          mk_ld(bb, 0), mk_ld(bb, 1)]
            else:
                # last pair: ctx chains only (stages appended by caller);
                # filler matmuls keep the PE duty high enough to hold the
                # HAM fast-clock state through the thin tail
                units += [c0[0], mk_fill(3), c0[1], mk_cn(0),
                          mk_fill(3), c1[0], mk_fill(3),
                          c1[1], mk_cn(1), mk_fill(3),
                          c2[0], mk_fill(3), c2[1], mk_cn(2),
                          c3[0], c3[1], mk_cn(3),
                          (20, dfn, 0), mk_fill(6)]
            return units, o_prev

        for p in range(1, NP):
            pts = [ep.tile([128, 2 * S], BF16, tag="e", name="e")
                   for _ in range(JP)]
            exps = mk_exps(pts)
            units, o_prev = build_units(p)
            if p == NP - 1:
                # last pair: stage ctx for j-tiles 0..11 early; gated on
                # the ACT groups that emit the needed exps (3 groups/jp)
                for stage, jts, ming in ((0, range(0, 8), 12),
                                         (1, range(8, 12), 18),
                                         (2, range(12, 14), 21)):
                    for g in range(4):
                        gate = ming + g // 2
                        cstate = {"ps": None, "n": 0,
                                  "tot": (32, 16, 8)[stage]}

                        def sfn(g=g, stage=stage, jts=jts, cstate=cstate,
                                exps=exps, p=p):
                            ps = ctx_quartet(p, 4 * g, exps, list(jts), cstate)
                            if stage == 0:
                                cst = op.tile(
                                    [128, 4 * (D + 1)], F32, tag="cst",
                                    bufs=4, name="cst",
                                )
                                nc.vector.tensor_copy(out=cst[:], in_=ps[:])
                                cst_tiles.append(cst)
                            else:
                                nc.vector.tensor_add(
                                    out=cst_tiles[g][:], in0=ps[:],
                                    in1=cst_tiles[g][:],
                                )
                        units.append(((1120, 580, 300)[stage], sfn, gate))

            uidx = {"i": 0}

            def pop_units(budget, cur_g):
                spent = 0
                while uidx["i"] < len(units):
                    est, fn, min_g = units[uidx["i"]]
                    if min_g > cur_g:
                        break
                    if spent > 0 and spent + est > budget:
                        break
                    fn()
                    spent += est
                    uidx["i"] += 1

            # budget the unit supply across the 24 pop points so the PE
            # never runs dry late in the pair; pops sized to the ACT
            # window that follows them (wide 1857 ns, narrow ~1050).
            # psn3 -> ACT_nb is the tight deadline: everything popped at
            # bw/bn1 sits in front of psn3 in the PE queue, so keep bn1
            # near zero and push the volume to bn2 (whose followers have
            # the whole next wide-ACT window of slack).
            E = sum(u[0] for u in units)
            boost = 1.4 if p == NP - 1 else 1.1
            bw = max(1200, int(E * 0.42 * boost / JP))
            bn1 = 150
            bn2 = max(1200, int(E * 0.58 * boost / JP))

            # Emission order keeps the PE fed AHEAD of the Scalar engine,
            # and never puts a dependency-blocked matmul in front of
            # runnable work in the in-order PE queue:
            #   ACT_w | pop | ACT_na | pop | W'(jp+1) | N3 | ACT_nb |
            #   pop | N2'(jp+1) | ...
            # W' rides the na window (psW freed one group earlier); N3
            # must wait na's end (psN single slot) so it goes last; N2'
            # waits nb's end and runs under ACT_w(jp+1).
            g = 0
            if pend_scores is not None:
                psw, psn2 = pend_scores
            else:
                psw = emit_W_mms(p, 0)
                psn2 = emit_N_mms(p, 0, 2)
            for jp in range(JP):
                emit_W_act(psw, pts[jp])
                g += 1
                pop_units(bw, g)
                emit_N_act(psn2, 2, pts[jp])
                g += 1
                pop_units(bn1, g)
                psn3 = emit_N_mms(p, jp, 3)
                if jp + 1 < JP:
                    psw = emit_W_mms(p, jp + 1)
                emit_N_act(psn3, 3, pts[jp])
                g += 1
                pop_units(bn2, g)
                if jp + 1 < JP:
                    psn2 = emit_N_mms(p, jp + 1, 2)
            # prefetch the next pair's first scores matmuls so its wide
            # ACT starts right at this pair's last narrow-ACT end; the
            # leftover-unit drain rides behind/between them
            if p + 1 < NP:
                psw_n = emit_W_mms(p + 1, 0)
                pop_units(1 << 30, 24)
                psn2_n = emit_N_mms(p + 1, 0, 2)
                pend_scores = (psw_n, psn2_n)
            else:
                pop_units(1 << 30, 24)
                pend_scores = None
            prev = (p, exps)

        # =================== epilogue ===================
        # last pair: j-tiles 14,15 + combine + normalize + out
        p, exps = prev
        o_last = op.tile([128, IT, D], F32, tag="o", name="o_last")
        for g in range(4):
            cstate = {"ps": None, "n": 0, "tot": 8}
            ps = ctx_quartet(p, 4 * g, exps, range(14, JT), cstate)
            nc.vector.tensor_add(
                out=cst_tiles[g][:], in0=ps[:], in1=cst_tiles[g][:]
            )
            ctx_norm(cst_tiles[g], 4 * g, o_last)
            emit_out_dma(p, o_last, groups=(g,))

    nc.compile()
    return nc


def _get_nc():
    if "nc" not in _CACHE:
        _CACHE["nc"] = _build()
    return _CACHE["nc"]


def kernel(
    query,
    key=None,
    attention_mask=None,
    Wq=None,
    bq=None,
    Wk=None,
    bk=None,
    Wv=None,
    bv=None,
    seq_length=2048,
    **_unused,
):
    query = np.asarray(query)
    Wq = np.asarray(Wq)
    Wk = np.asarray(Wk)
    Wv = np.asarray(Wv)
    if attention_mask is not None and not np.all(np.asarray(attention_mask) == 1):
        raise NotImplementedError("kernel assumes an all-ones attention mask")
    for bias in (bq, bk, bv):
        if bias is not None and np.any(np.asarray(bias)):
            raise NotImplementedError("kernel assumes zero biases")

    x = query.reshape(-1, HID)  # [8192, 1024]
    xt = np.ascontiguousarray(x.T).astype(ml_dtypes.bfloat16)  # [1024, 8192]

    in_maps = []
    for c in range(NCORES):
        cols = slice(c * 128, (c + 1) * 128)
        in_maps.append(
            {
                "xt": xt,
                "wq": np.ascontiguousarray(Wq[:, cols]).astype(ml_dtypes.bfloat16),
                "wk": np.ascontiguousarray(Wk[:, cols]).astype(ml_dtypes.bfloat16),
                "wv": np.ascontiguousarray(Wv[:, cols]).astype(ml_dtypes.bfloat16),
            }
        )

    nc = _get_nc()
    res = run_bass_kernel_spmd(
        nc,
        in_maps,
        core_ids=list(range(NCORES)),
        trace=bool(_CACHE.get("trace", False)),
    )
    _CACHE["last_result"] = res
    out = np.concatenate(
        [res.results[c]["out"] for c in range(NCORES)], axis=1
    ).astype(np.float32)
    return out


# revision 59
# speedup vs baseline: 1.0369x; 1.0369x over previous
"""Trainium2 Bass kernel for nn_Attention_27797028340174.

Multi-head attention, B=4, S=2048, H=16 heads, D=64 (HID=1024):
    x = query.reshape(B*S, HID)                     (the `key` input is
    q,k,v = x@Wq+bq, x@Wk+bk, x@Wv+bv                ignored: source bug
    per (b,h): softmax(q k^T / 8) @ v                makes k,v from query)

Sharding: tensor-parallel over the 16 heads -> 2 heads per NeuronCore,
zero collectives. Each core receives the full transposed activations
xT = x.T (bf16) plus its 128-column slice of Wq/Wk/Wv, and produces its
[8192, 128] slice of the output; the host concatenates slices.

v4 design notes (delta vs v3). The scores matmuls contract over D=64
only; v3 padded them to K=128 (half the PE array multiplied zeros).
v4 runs them as 64-row matmuls in ALTERNATING partition halves: even
j-tiles' kT live in SBUF partitions 0:63, odd j-tiles' in 64:127, and
qT is duplicated into both halves. Consecutive even/odd matmuls then
occupy disjoint PE row-groups and overlap almost fully (measured
~110 ns per 512-wide MM vs ~240 ns unpaired), halving scores PE time.
Per j-tile-PAIR (jp): ic0/ic1 of both parities interleave into one
[128,2048] psum (each 512-chunk = one bank) consumed by ONE wide exp
ACT whose 4D out-AP de-interleaves into the pair's exp tile; ic2/ic3
go through two [128,1024] narrow psums/ACTs. ACT count and PSUM bank
budget match v3 (the Scalar engine exp stream stays the pacer at
~32 us/pair); the PE gains ~8 us/pair of slack which absorbs the
ctx/proj/v units without fills running the clock-warmup risk.

Device algorithm per core (all matmuls bf16, fp32 PSUM):
  qT,kT = W.T @ xT   transposed activations, 64 valid rows per parity
  v     = xT.T @ Wv  [seq, 128] natural layout (+ ones column)
  per (b,h), per jp: scoresT = kT_half.T @ qT_half  (K=64, paired)
      expT = exp(scoresT/8)  (ScalarE, bf16 out)
  per i-group of 4 tiles: ctx, Z = expT.T @ [v | 1]  (K=128 chains)
      out = ctx * reciprocal(Z)    (VectorE)

Assumptions hard-verified on host: attention_mask all ones (mask term
== 0), zero biases. These hold for the problem's setup_inputs().
"""

from contextlib import ExitStack

import numpy as np
import ml_dtypes

import concourse.bass as bass
import concourse.tile as tile
from concourse import bacc, mybir
from concourse.bass_utils import run_bass_kernel_spmd

BF16 = mybir.dt.bfloat16
F32 = mybir.dt.float32

B = 4  # batches
S = 2048  # seq per batch
S2 = 1024  # kT cols per head (16 j-tiles packed as 8 parity pairs)
HID = 1024
NCORES = 8
NH = 2  # heads per core
D = 64
KT = 8  # hid tiles of 128
JT = 16  # key tiles of 128 per batch
JP = 8  # key tile pairs per batch
IT = 16  # query tiles of 128 per batch
CH = 4  # seq chunks of 512 per batch
CW = 512  # chunk width

EXP_BUFS = 16  # [128, 2*S] pair tiles
XT_BUFS = 3

_CACHE = {}


def _build():
    nc = bacc.Bacc(
        "TRN2", target_bir_lowering=False, debug=False, num_devices=NCORES
    )
    xt = nc.dram_tensor("xt", [HID, B * S], BF16, kind="ExternalInput")
    wq = nc.dram_tensor("wq", [HID, 128], BF16, kind="ExternalInput")
    wk = nc.dram_tensor("wk", [HID, 128], BF16, kind="ExternalInput")
    wv = nc.dram_tensor("wv", [HID, 128], BF16, kind="ExternalInput")
    out = nc.dram_tensor("out", [B * S, 128], F32, kind="ExternalOutput")

    xt_v = xt.ap().rearrange("(kt p) n -> p kt n", p=128)  # [128, 8, 8192]
    out_v = out.ap().rearrange("(b it p) c -> p b it c", it=IT, p=128)

    with tile.TileContext(nc) as tc, ExitStack() as ctx:
        wp = ctx.enter_context(tc.tile_pool(name="w", bufs=1))
        xp = ctx.enter_context(tc.tile_pool(name="x", bufs=XT_BUFS))
        qkp = ctx.enter_context(tc.tile_pool(name="qk", bufs=2))
        ep = ctx.enter_context(tc.tile_pool(name="e", bufs=EXP_BUFS))
        op = ctx.enter_context(tc.tile_pool(name="o", bufs=2))
        zp = ctx.enter_context(tc.tile_pool(name="z", bufs=4))
        ksp = ctx.enter_context(tc.tile_pool(name="ks", bufs=3))
        # 8 PSUM banks: scores wide [128,2048] (4) + narrow [128,1024] (2)
        # + proj/v chain bank (1) + ctx chain bank (1).  ctx chains stay
        # open across other units, and a chain start= clears its whole
        # 2KB bank -- so ctx must NEVER share a bank with proj/v chains.
        psW = ctx.enter_context(tc.tile_pool(name="psW", bufs=1, space="PSUM"))
        psN = ctx.enter_context(tc.tile_pool(name="psN", bufs=1, space="PSUM"))
        psq = ctx.enter_context(tc.tile_pool(name="psq", bufs=1, space="PSUM"))
        psc = ctx.enter_context(tc.tile_pool(name="psc", bufs=1, space="PSUM"))

        wq_sb = wp.tile([128, KT, 128], BF16)
        nc.sync.dma_start(wq_sb[:], wq.ap().rearrange("(kt p) m -> p kt m", p=128))
        wk_sb = wp.tile([128, KT, 128], BF16)
        wv_sb = wp.tile([128, KT, 128], BF16)
        w_loaded = {"k": False, "v": False}

        def load_w(which):
            if not w_loaded[which]:
                src = wk if which == "k" else wv
                dst = wk_sb if which == "k" else wv_sb
                eng = nc.scalar if which == "k" else nc.sync
                eng.dma_start(
                    dst[:], src.ap().rearrange("(kt p) m -> p kt m", p=128)
                )
                w_loaded[which] = True

        state = {}
        xt_carry = {}  # (batch, chunk) -> in-flight xt SBUF tile

        # qT: [128, NH*S], head h at cols h*S, rows 0:64 and 64:128 hold
        # the SAME data (duplicated so even/odd scores matmuls can read
        # from either partition half).  kT: [128, NH*S2], head h at cols
        # h*S2; j-tile pair jp at cols jp*128; rows 0:64 = even j-tile
        # (2jp), rows 64:128 = odd j-tile (2jp+1).
        qT_bufs = [wp.tile([128, NH * S], BF16, name=f"qTb{i}") for i in (0, 1)]
        kT_bufs = [wp.tile([128, NH * S2], BF16, name=f"kTb{i}") for i in (0, 1)]

        def alloc_batch(b):
            st = {}
            st["qT"] = qT_bufs[b % 2]
            st["kT"] = kT_bufs[b % 2]
            st["v"] = qkp.tile([128, JT, NH, D + 1], BF16, tag="v", name="v")
            nc.vector.memset(st["v"][:, :, :, D], 1.0)
            state[b] = st

        # ---- projection emitters ----
        def _q_copyout(b, ch, ps, ring=None):
            qT = state[b]["qT"]
            eng = nc.scalar if ring == "scalar" else nc.sync
            c0 = slice(ch * CW, (ch + 1) * CW)
            c1 = slice(S + ch * CW, S + (ch + 1) * CW)
            nc.vector.tensor_copy(out=qT[0:64, c0], in_=ps[0:64, :])
            eng.dma_start(qT[64:128, c0], qT[0:64, c0])
            nc.vector.tensor_copy(out=qT[64:128, c1], in_=ps[64:128, :])
            eng.dma_start(qT[0:64, c1], qT[64:128, c1])

        def _k_copyout(b, ch, ps, ring=None):
            kT = state[b]["kT"]
            eng = nc.scalar if ring == "scalar" else nc.sync
            base = 2 * ch * 128  # 256-col pair block within a head region
            psv0 = ps[0:64, :].rearrange("p (jp pr c) -> p jp pr c", pr=2, c=128)
            psv1 = ps[64:128, :].rearrange("p (jp pr c) -> p jp pr c", pr=2, c=128)
            stg = ksp.tile([128, 256], BF16, tag="kstg", name="kstg")
            k0e = kT[0:64, base : base + 256].rearrange(
                "p (jp c) -> p jp c", c=128
            )
            nc.vector.tensor_copy(out=k0e, in_=psv0[:, :, 0])
            stg0 = stg[0:64, :].rearrange("p (jp c) -> p jp c", c=128)
            nc.vector.tensor_copy(out=stg0, in_=psv0[:, :, 1])
            eng.dma_start(kT[64:128, base : base + 256], stg[0:64, :])
            k1o = kT[64:128, S2 + base : S2 + base + 256].rearrange(
                "p (jp c) -> p jp c", c=128
            )
            nc.vector.tensor_copy(out=k1o, in_=psv1[:, :, 1])
            stg1 = stg[64:128, :].rearrange("p (jp c) -> p jp c", c=128)
            nc.vector.tensor_copy(out=stg1, in_=psv1[:, :, 0])
            eng.dma_start(kT[0:64, S2 + base : S2 + base + 256], stg[64:128, :])

        def load_xt(b, ch, ring=None):
            gc = b * CH + ch
            xt_t = xp.tile([128, KT, CW], BF16, tag="xt", name="xt_t")
            if ring == "split":
                # halve latency: kt 0:4 on the sync ring, 4:8 on scalar
                nc.sync.dma_start(
                    xt_t[:, 0:4], xt_v[:, 0:4, gc * CW : (gc + 1) * CW]
                )
                nc.scalar.dma_start(
                    xt_t[:, 4:KT], xt_v[:, 4:KT, gc * CW : (gc + 1) * CW]
                )
                return xt_t
            eng = nc.scalar if ring == "scalar" else nc.sync
            eng.dma_start(xt_t[:], xt_v[:, :, gc * CW : (gc + 1) * CW])
            return xt_t

        def proj_half1(b, ch, which, xt_t, pool=None):
            """First half of a q/k projection chunk: open the chain."""
            if which == "k":
                load_w("k")
            w_sb = wq_sb if which == "q" else wk_sb
            ps = (pool or psq).tile(
                [128, CW], F32,
                tag="aux" if (pool or psq) is psq else "ctx", name="ps_qk")
            for kt in range(4):
                nc.tensor.matmul(
                    ps[:], lhsT=w_sb[:, kt], rhs=xt_t[:, kt],
                    start=(kt == 0), stop=False,
                )
            return ps

        def proj_half2(b, ch, which, xt_t, ps, ring=None):
            w_sb = wq_sb if which == "q" else wk_sb
            for kt in range(4, KT):
                nc.tensor.matmul(
                    ps[:], lhsT=w_sb[:, kt], rhs=xt_t[:, kt],
                    start=False, stop=(kt == KT - 1),
                )
            if which == "q":
                _q_copyout(b, ch, ps, ring=ring)
            else:
                _k_copyout(b, ch, ps, ring=ring)

        def emit_proj(b, ch, which, xt_t, pool=None, ring=None):
            """Full q or k projection chunk (8 matmuls + copyout)."""
            ps = proj_half1(b, ch, which, xt_t, pool=pool)
            proj_half2(b, ch, which, xt_t, ps, ring=ring)

        def emit_v_half(b, ch, half, xt_t):
            load_w("v")
            st = state[b]
            ps_full = psq.tile([128, CW], F32, tag="aux", name="ps_v")
            ps = ps_full[:, 0:256]
            for si, sub in enumerate((2 * half, 2 * half + 1)):
                for kt in range(KT):
                    nc.tensor.matmul(
                        ps[:, si * 128 : (si + 1) * 128],
                        lhsT=xt_t[:, kt, sub * 128 : (sub + 1) * 128],
                        rhs=wv_sb[:, kt],
                        start=(si == 0 and kt == 0),
                        stop=(si == 1 and kt == KT - 1),
                    )
            nc.vector.tensor_copy(
                out=st["v"][:, ch * 4 + 2 * half : ch * 4 + 2 * half + 2, :, 0:D],
                in_=ps[:].rearrange("p (s h d) -> p s h d", s=2, h=NH),
            )

        # ---- scores emitters ----
        def _smm(p, jt, ic, ps_dst):
            b, h = divmod(p, NH)
            st = state[b]
            jp, pr = divmod(jt, 2)
            half = slice(0, 64) if pr == 0 else slice(64, 128)
            nc.tensor.matmul(
                ps_dst,
                lhsT=st["kT"][half, h * S2 + jp * 128 : h * S2 + (jp + 1) * 128],
                rhs=st["qT"][half, h * S + ic * CW : h * S + (ic + 1) * CW],
                start=True,
                stop=True,
            )

        def _act(e_slice, ps_slice):
            nc.scalar.activation(
                e_slice, ps_slice, mybir.ActivationFunctionType.Exp, scale=0.125
            )

        def _eo(e):
            # [128, 4*512 per parity] -> [p, ic(4), parity(2), 512]
            return e[:].rearrange("p (t a c) -> p a t c", t=2, a=4)

        def emit_W_mms(p, jp):
            """ics 0,1 of both parities interleaved into one [128,2048]."""
            psw = psW.tile([128, 2048], F32, tag="sW", name="ps_w")
            for ic in (0, 1):
                for pr in (0, 1):
                    _smm(p, 2 * jp + pr, ic,
                         psw[:, (2 * ic + pr) * CW : (2 * ic + pr + 1) * CW])
            return psw

        def emit_W_act(psw, e):
            inv = psw[:].rearrange("p (a t c) -> p a t c", a=2, t=2)
            _act(_eo(e)[:, 0:2], inv)

        def emit_N_mms(p, jp, ic, pool=None):
            pool = pool or psN
            psn = pool.tile([128, 1024], F32,
                            tag="sN" if pool is psN else "sW", name="ps_n")
            for pr in (0, 1):
                _smm(p, 2 * jp + pr, ic, psn[:, pr * CW : (pr + 1) * CW])
            return psn

        def emit_N_act(psn, ic, e):
            inv = psn[:].rearrange("p (t c) -> p t c", t=2)
            _act(_eo(e)[:, ic], inv)

        def jp_W(p, jp, e):
            emit_W_act(emit_W_mms(p, jp), e)

        pro_npool = {"i": 0}

        def jp_N(p, jp, ic, e):
            """Prologue narrow group; alternates the psN / psW banks so
            consecutive narrow ACTs never wait each other's psum."""
            pool = psN if pro_npool["i"] % 2 == 0 else psW
            pro_npool["i"] += 1
            emit_N_act(emit_N_mms(p, jp, ic, pool=pool), ic, e)

        def mk_exps(pts):
            return [pts[j // 2][:, (j % 2) * S : (j % 2) * S + S] for j in range(JT)]

        # ---- ctx emitters ----
        def ctx_quartet(p, it0, exps, jts, cstate):
            """4 j-tiles x 4 q-slices of an accumulation chain."""
            b, h = divmod(p, NH)
            st = state[b]
            W = D + 1
            if cstate["ps"] is None:
                cstate["ps"] = psc.tile([128, 4 * W], F32, tag="ctx", name="ps_c")
            ps = cstate["ps"]
            for jt in jts:
                for q in range(4):
                    cstate["n"] += 1
                    nc.tensor.matmul(
                        ps[:, q * W : (q + 1) * W],
                        lhsT=exps[jt][:, (it0 + q) * 128 : (it0 + q + 1) * 128],
                        rhs=st["v"][:, jt, h],
                        start=(cstate["n"] == 1),
                        stop=(cstate["n"] == cstate["tot"]),
                    )
            return ps

        def ctx_norm(ps, it0, o_sb):
            W = D + 1
            rz = zp.tile([128, 4], F32, tag="rz", name="rz")
            z_view = ps[:].rearrange("p (q w) -> p q w", w=W)[:, :, D]
            nc.vector.reciprocal(rz[:], z_view)
            for q in range(4):
                nc.vector.tensor_scalar_mul(
                    o_sb[:, it0 + q], ps[:, q * W : q * W + D], rz[:, q : q + 1]
                )

        def emit_out_dma(p, o_sb, groups=(0, 1, 2, 3)):
            b, h = divmod(p, NH)
            for g in groups:
                nc.sync.dma_start(
                    out_v[:, b, 4 * g : 4 * g + 4, h * D : (h + 1) * D],
                    o_sb[:, 4 * g : 4 * g + 4],
                )

        # =================== prologue ===================
        warm_src = wp.tile([128, 1], F32, name="warm_src")
        warm_dst = wp.tile([128, 1], F32, name="warm_dst")
        nc.vector.memset(warm_src[:], 0.0)
        nc.scalar.activation(
            warm_dst[:], warm_src[:], mybir.ActivationFunctionType.Exp
        )
        # PE clock ramp warmup while the first DMAs fly
        warm_w = wp.tile([128, 128], BF16, name="warm_w")
        nc.vector.memset(warm_w[:], 0.0)
        warm_r = wp.tile([128, CW], BF16, name="warm_r")
        nc.vector.memset(warm_r[:], 0.0)
        warm_ps_full = psq.tile([128, CW], F32, tag="aux", name="warm_ps")
        warm_ps = warm_ps_full[:, 0:64]
        NWARM = 64
        for r in range(NWARM):
            nc.tensor.matmul(
                warm_ps[:, 0:64],
                lhsT=warm_w[:],
                rhs=warm_r[:, 0:64],
                start=(r == 0),
                stop=(r == NWARM - 1),
            )
        warm_junk = wp.tile([128, 64], F32, name="warm_junk")
        nc.vector.tensor_copy(out=warm_junk[:], in_=warm_ps[:, 0:64])
        # second warm chain: keep the PE busy (HAM stays at 8/8) while
        # the xt0 DMA is still in flight; sized to end just before the
        # transfer completes (~16us in)
        warm_ps2_full = psc.tile([128, CW], F32, tag="ctx", name="warm_ps2")
        warm_ps2 = warm_ps2_full[:, 0:64]
        NWARM2 = 84
        for r in range(NWARM2):
            nc.tensor.matmul(
                warm_ps2[:, 0:64],
                lhsT=warm_w[:],
                rhs=warm_r[:, 0:64],
                start=(r == 0),
                stop=(r == NWARM2 - 1),
            )
        nc.vector.tensor_copy(out=warm_junk[:], in_=warm_ps2[:, 0:64])

        alloc_batch(0)
        pts0 = [ep.tile([128, 2 * S], BF16, tag="e", name="e0") for _ in range(JP)]
        exps0 = mk_exps(pts0)

        # batch-0 projections laddered with pair-0 scores; alternate the
        # chain bank psq/psc so chain N+1 never waits chain N's copyout
        pro_pool = {"i": 0}

        def pproj(b, ch, which, xt_t, ring=None):
            pool = psq if pro_pool["i"] % 2 == 0 else psc
            pro_pool["i"] += 1
            emit_proj(b, ch, which, xt_t, pool=pool, ring=ring)

        # ladder: initial DMAs fly on four separate queues (xt0 sync,
        # wq scalar, wk vector, xt1 tensor); the first scores land as
        # narrow 1024-wide ACTs as soon as q0+k0 are projected, so the
        # Scalar engine starts its exp stream ~15us earlier than a
        # wide-first ladder would allow.
        xt_t0 = load_xt(0, 0, ring="split")
        load_w("k")
        xt_t1 = load_xt(0, 1, ring="split")
        pproj(0, 0, "q", xt_t0)
        pproj(0, 0, "k", xt_t0)
        jp_N(0, 0, 0, pts0[0])
        jp_N(0, 1, 0, pts0[1])
        pproj(0, 1, "q", xt_t1)
        jp_N(0, 0, 1, pts0[0])
        jp_N(0, 1, 1, pts0[1])
        xt_t2 = load_xt(0, 2)
        pproj(0, 1, "k", xt_t1)
        jp_N(0, 2, 0, pts0[2])
        jp_N(0, 3, 0, pts0[3])
        pproj(0, 2, "q", xt_t2)
        jp_N(0, 2, 1, pts0[2])
        jp_N(0, 3, 1, pts0[3])
        jp_N(0, 0, 2, pts0[0])
        pproj(0, 2, "k", xt_t2)
        jp_N(0, 1, 2, pts0[1])
        jp_N(0, 2, 2, pts0[2])
        xt_t3 = load_xt(0, 3, ring="tensor")
        jp_W(0, 4, pts0[4])
        pproj(0, 3, "q", xt_t3)
        jp_N(0, 3, 2, pts0[3])
        jp_N(0, 4, 2, pts0[4])
        jp_N(0, 0, 3, pts0[0])
        pproj(0, 3, "k", xt_t3)
        jp_N(0, 1, 3, pts0[1])
        jp_N(0, 2, 3, pts0[2])
        jp_N(0, 3, 3, pts0[3])
        load_w("v")
        xtv0 = load_xt(0, 0, ring="scalar")
        jp_W(0, 5, pts0[5])
        jp_N(0, 4, 3, pts0[4])
        jp_N(0, 5, 2, pts0[5])
        xtv1 = load_xt(0, 1, ring="scalar")
        emit_v_half(0, 0, 0, xtv0)
        jp_W(0, 6, pts0[6])
        emit_v_half(0, 0, 1, xtv0)
        jp_N(0, 5, 3, pts0[5])
        jp_N(0, 6, 2, pts0[6])
        xtv2 = load_xt(0, 2, ring="scalar")
        emit_v_half(0, 1, 0, xtv1)
        jp_W(0, 7, pts0[7])
        emit_v_half(0, 1, 1, xtv1)
        jp_N(0, 6, 3, pts0[6])
        jp_N(0, 7, 2, pts0[7])
        xtv3 = load_xt(0, 3, ring="scalar")
        emit_v_half(0, 2, 0, xtv2)
        jp_N(0, 7, 3, pts0[7])
        emit_v_half(0, 2, 1, xtv2)
        # pair 1's units consume these: v(0,3) halves + batch-1 chunk-0/1
        xt_carry[(0, 3)] = xtv3
        xt_carry[(1, 0)] = load_xt(1, 0, ring="scalar")
        xt_carry[(1, 1)] = load_xt(1, 1, ring="scalar")
        prev = (0, exps0)

        # =================== steady pairs ===================
        NP = B * NH
        cst_tiles = []
        pend_scores = None

        def build_units(p):
            """PE work units (est_ns, fn, min_grp) for pair p's slots.
            Units are fine-grained (ctx quartets ~650ns, projection
            halves ~880ns) so a pop between ACT groups never injects a
            multi-us block of foreign matmuls ahead of the next scores
            matmuls in the PE stream.  Constraints encoded in the hand
            ordering: a proj chain's two halves have no other psq user
            between them (psq is single banked); consecutive psq chains
            are separated by a ctx quartet so chain N+1's start never
            waits on chain N's copyout; ctx chain g+1 opens only after
            chain g's norm (they share the psc bank); each xt DMA is
            issued >=4 units (and usually a pair boundary) before its
            consumer so an in-flight transfer never head-of-line blocks
            the in-order PE queue."""
            b, h = divmod(p, NH)
            pp, pexps = prev
            o_prev = op.tile([128, IT, D], F32, tag="o", name="o_sb")

            cstates = [{"ps": None, "n": 0, "tot": 64} for _ in range(4)]

            def mk_cq(g, jq):
                def cfn(g=g, jq=jq):
                    ctx_quartet(pp, 4 * g, pexps,
                                range(4 * jq, 4 * jq + 4), cstates[g])
                return (660, cfn, 0)

            def mk_cn(g):
                def nfn(g=g):
                    ctx_norm(cstates[g]["ps"], 4 * g, o_prev)
                return (90, nfn, 0)

            def dfn():
                emit_out_dma(pp, o_prev)

            pschain = {}

            def mk_ld(bb, chn):
                def ld(bb=bb, chn=chn):
                    xt_carry[(bb, chn)] = load_xt(bb, chn)
                return (30, ld, 0)

            def mk_vh(bb, chn, half):
                def vh(bb=bb, chn=chn, half=half):
                    emit_v_half(bb, chn, half, xt_carry[(bb, chn)])
                return (890, vh, 0)

            def mk_qk1(bb, chn, which):
                def pj(bb=bb, chn=chn, which=which):
                    pschain[which] = proj_half1(
                        bb, chn, which, xt_carry[(bb, chn)]
                    )
                return (880, pj, 0)

            def mk_qk2(bb, chn, which):
                def pj(bb=bb, chn=chn, which=which):
                    proj_half2(bb, chn, which, xt_carry[(bb, chn)],
                               pschain[which])
                return (880, pj, 0)

            def mk_alloc(bb):
                def al(bb=bb):
                    alloc_batch(bb)
                return (10, al, 0)

            def mk_fill(n):
                def fl(n=n):
                    ps = psq.tile([128, CW], F32, tag="aux", name="ps_fill")
                    for r in range(n):
                        nc.tensor.matmul(
                            ps[:],
                            lhsT=warm_w[:],
                            rhs=warm_r[:],
                            start=(r == 0),
                            stop=(r == n - 1),
                        )
                return (n * 215, fl, 0)

            def Cq(g):
                return [mk_cq(g, jq) for jq in range(4)]

            c0, c1, c2, c3 = Cq(0), Cq(1), Cq(2), Cq(3)
            units = []
            if h == 0 and b >= 1:
                # v(b) all chunks + qk(b+1) chunk0; (b,0) and (b,1) were
                # prefetched by the previous pair's tail
                units += [mk_vh(b, 0, 0), c0[0], mk_vh(b, 0, 1), c0[1],
                          mk_ld(b, 2),
                          mk_vh(b, 1, 0), c0[2], mk_vh(b, 1, 1), c0[3],
                          mk_cn(0),
                          mk_ld(b, 3),
                          mk_vh(b, 2, 0), c1[0], mk_vh(b, 2, 1), c1[1]]
                if b + 1 < B:
                    units += [mk_alloc(b + 1), mk_ld(b + 1, 0)]
                units += [mk_vh(b, 3, 0), c1[2], mk_vh(b, 3, 1), c1[3],
                          mk_cn(1)]
                if b + 1 < B:
                    units += [mk_qk1(b + 1, 0, "q"), c2[0],
                              mk_qk2(b + 1, 0, "q"), c2[1], c2[2],
                              mk_qk1(b + 1, 0, "k"), c2[3],
                              mk_qk2(b + 1, 0, "k"), mk_cn(2),
                              c3[0], c3[1], c3[2], c3[3], mk_cn(3),
                              (20, dfn, 0),
                              # prefetch c1 for the following h1 pair
                              mk_ld(b + 1, 1), mk_fill(4)]
                else:
                    units += [c2[0], c2[1], mk_fill(4), c2[2], c2[3],
                              mk_cn(2), mk_fill(4),
                              c3[0], c3[1], mk_fill(4), c3[2], c3[3],
                              mk_cn(3), (20, dfn, 0), mk_fill(4)]
            elif h == 1 and b + 1 < B:
                bb = b + 1
                if b == 0:
                    # pair 1 absorbs the prologue tail: v(0,3) halves,
                    # batch-1 alloc and its chunk-0 projections
                    units += [mk_vh(0, 3, 0), c0[0], mk_vh(0, 3, 1),
                              c0[1], mk_alloc(1),
                              mk_qk1(1, 0, "q"), c0[2],
                              mk_qk2(1, 0, "q"), c0[3], mk_cn(0),
                              mk_qk1(1, 0, "k"), c1[0],
                              mk_qk2(1, 0, "k"), c1[1]]
                else:
                    units += [c0[0], c0[1], c0[2], c0[3], mk_cn(0),
                              c1[0], c1[1]]
                # chunks 1-3 of batch bb; (bb,1) prefetched earlier
                units += [mk_qk1(bb, 1, "q"), mk_ld(bb, 2), c1[2],
                          mk_qk2(bb, 1, "q"), c1[3], mk_cn(1),
                          mk_qk1(bb, 1, "k"), c2[0],
                          mk_qk2(bb, 1, "k"), c2[1],
                          mk_qk1(bb, 2, "q"), mk_ld(bb, 3), c2[2],
                          mk_qk2(bb, 2, "q"), c2[3], mk_cn(2),
                          mk_qk1(bb, 2, "k"), c3[0],
                          mk_qk2(bb, 2, "k"), c3[1],
                          mk_qk1(bb, 3, "q"), c3[2],
                          mk_qk2(bb, 3, "q"), c3[3],
                          mk_qk1(bb, 3, "k"), mk_cn(3),
                          mk_qk2(bb, 3, "k"),
                          (20, dfn, 0),
                          # prefetch v0/v1 chunks for the next h0 pair
                          mk_ld(bb, 0), mk_ld(bb, 1), mk_fill(2)]
            else:
                # last pair: ctx chains only (stages appended by caller);
                # filler matmuls keep the PE duty high enough to hold the
                # HAM fast-clock state through the thin tail
                units += [c0[0], c0[1], mk_fill(3), c0[2], c0[3], mk_cn(0),
                          mk_fill(3), c1[0], c1[1], mk_fill(3), c1[2],
                          c1[3], mk_cn(1), mk_fill(3),
                          c2[0], c2[1], mk_fill(3), c2[2], c2[3], mk_cn(2),
                          c3[0], c3[1], c3[2], c3[3], mk_cn(3),
                          (20, dfn, 0), mk_fill(6)]
            return units, o_prev

        for p in range(1, NP):
            pts = [ep.tile([128, 2 * S], BF16, tag="e", name="e")
                   for _ in range(JP)]
            exps = mk_exps(pts)
            units, o_prev = build_units(p)
            if p == NP - 1:
                # last pair: stage ctx for j-tiles 0..11 early; gated on
                # the ACT groups that emit the needed exps (3 groups/jp)
                for stage, jts, ming in ((0, range(0, 8), 12),
                                         (1, range(8, 12), 18),
                                         (2, range(12, 14), 21)):
                    for g in range(4):
                        gate = ming + g // 2
                        cstate = {"ps": None, "n": 0,
                                  "tot": (32, 16, 8)[stage]}

                        def sfn(g=g, stage=stage, jts=jts, cstate=cstate,
                                exps=exps, p=p):
                            ps = ctx_quartet(p, 4 * g, exps, list(jts), cstate)
                            if stage == 0:
                                cst = op.tile(
                                    [128, 4 * (D + 1)], F32, tag="cst",
                                    bufs=4, name="cst",
                                )
                                nc.vector.tensor_copy(out=cst[:], in_=ps[:])
                                cst_tiles.append(cst)
                            else:
                                nc.vector.tensor_add(
                                    out=cst_tiles[g][:], in0=ps[:],
                                    in1=cst_tiles[g][:],
                                )
                        units.append(((1120, 580, 300)[stage], sfn, gate))

            uidx = {"i": 0}

            def pop_units(budget, cur_g):
                spent = 0
                while uidx["i"] < len(units):
                    est, fn, min_g = units[uidx["i"]]
                    if min_g > cur_g:
                        break
                    if spent > 0 and spent + est > budget:
                        break
                    fn()
                    spent += est
                    uidx["i"] += 1

            # budget the unit supply across the 24 pop points so the PE
            # never runs dry late in the pair; pops sized to the ACT
            # window that follows them (wide 1857 ns, narrow ~1050).
            # psn3 -> ACT_nb is the tight deadline: everything popped at
            # bw/bn1 sits in front of psn3 in the PE queue, so keep bn1
            # near zero and push the volume to bn2 (whose followers have
            # the whole next wide-ACT window of slack).
            E = sum(u[0] for u in units)
            boost = 1.4 if p == NP - 1 else 1.1
            bw = max(1200, int(E * 0.42 * boost / JP))
            bn1 = 150
            bn2 = max(1200, int(E * 0.58 * boost / JP))

            # Emission order keeps the PE fed AHEAD of the Scalar engine,
            # and never puts a dependency-blocked matmul in front of
            # runnable work in the in-order PE queue:
            #   ACT_w | pop | ACT_na | pop | W'(jp+1) | N3 | ACT_nb |
            #   pop | N2'(jp+1) | ...
            # W' rides the na window (psW freed one group earlier); N3
            # must wait na's end (psN single slot) so it goes last; N2'
            # waits nb's end and runs under ACT_w(jp+1).
            g = 0
            psw = emit_W_mms(p, 0)
            psn2 = emit_N_mms(p, 0, 2)
            for jp in range(JP):
                emit_W_act(psw, pts[jp])
                g += 1
                pop_units(bw, g)
                emit_N_act(psn2, 2, pts[jp])
                g += 1
                pop_units(bn1, g)
                psn3 = emit_N_mms(p, jp, 3)
                if jp + 1 < JP:
                    psw = emit_W_mms(p, jp + 1)
                emit_N_act(psn3, 3, pts[jp])
                g += 1
                pop_units(bn2, g)
                if jp + 1 < JP:
                    psn2 = emit_N_mms(p, jp + 1, 2)
            # drain remaining units
            pop_units(1 << 30, 24)
            prev = (p, exps)

        # =================== epilogue ===================
        # last pair: j-tiles 14,15 + combine + normalize + out
        p, exps = prev
        o_last = op.tile([128, IT, D], F32, tag="o", name="o_last")
        for g in range(4):
            cstate = {"ps": None, "n": 0, "tot": 8}
            ps = ctx_quartet(p, 4 * g, exps, range(14, JT), cstate)
            nc.vector.tensor_add(
                out=cst_tiles[g][:], in0=ps[:], in1=cst_tiles[g][:]
            )
            ctx_norm(cst_tiles[g], 4 * g, o_last)
            emit_out_dma(p, o_last, groups=(g,))

    nc.compile()
    return nc


def _get_nc():
    if "nc" not in _CACHE:
        _CACHE["nc"] = _build()
    return _CACHE["nc"]


def kernel(
    query,
    key=None,
    attention_mask=None,
    Wq=None,
    bq=None,
    Wk=None,
    bk=None,
    Wv=None,
    bv=None,
    seq_length=2048,
    **_unused,
):
    query = np.asarray(query)
    Wq = np.asarray(Wq)
    Wk = np.asarray(Wk)
    Wv = np.asarray(Wv)
    if attention_mask is not None and not np.all(np.asarray(attention_mask) == 1):
        raise NotImplementedError("kernel assumes an all-ones attention mask")
    for bias in (bq, bk, bv):
        if bias is not None and np.any(np.asarray(bias)):
            raise NotImplementedError("kernel assumes zero biases")

    x = query.reshape(-1, HID)  # [8192, 1024]
    xt = np.ascontiguousarray(x.T).astype(ml_dtypes.bfloat16)  # [1024, 8192]

    in_maps = []
    for c in range(NCORES):
        cols = slice(c * 128, (c + 1) * 128)
        in_maps.append(
            {
                "xt": xt,
                "wq": np.ascontiguousarray(Wq[:, cols]).astype(ml_dtypes.bfloat16),
                "wk": np.ascontiguousarray(Wk[:, cols]).astype(ml_dtypes.bfloat16),
                "wv": np.ascontiguousarray(Wv[:, cols]).astype(ml_dtypes.bfloat16),
            }
        )

    nc = _get_nc()
    res = run_bass_kernel_spmd(
        nc,
        in_maps,
        core_ids=list(range(NCORES)),
        trace=bool(_CACHE.get("trace", False)),
    )
    _CACHE["last_result"] = res
    out = np.concatenate(
        [res.results[c]["out"] for c in range(NCORES)], axis=1
    ).astype(np.float32)
    return out
